# revision 27
# baseline (speedup 1.0000x reference)
"""Causal self-attention (with the reference's inverted mask) on 8 TRN2
NeuronCores.

Problem (hardcoded): B=2, S=2048, D=1024, H=16 heads, head_dim=64, fp32.
  q/k/v = x @ W* + b*;  score = q k^T / 8;  score += tril(ones)*(-1e9)
  (inverted causal mask: the LOWER triangle incl. diagonal is masked, so
  softmax attends strictly to k > q; row q=S-1 is fully masked -> exactly
  uniform; recomputed on the host).  out = softmax(score) @ v @ Wo + bo.

Sharding: core c handles batch b = c//4 and heads [4*(c%4), 4*(c%4)+4).
Each core computes a partial output (its 4 heads' slice of attn @ Wo);
the host sums 4 partials per batch and adds bo.

v2 design (bf16 matmuls; rel-err budget 2e-2, measured ~4e-3):
  All matmul operands bf16 (fp32 PSUM accumulation).  bf16 enables the
  PE fast-weight-load path (f32r disabled it), halves DMA bytes and
  DVE element traffic.  ScalarE runs ONLY the exps (its throughput is
  1 elem/lane/cycle regardless of dtype and it is the phase-B floor);
  all PSUM evacuations moved to the DVE (tensor_scalar_add with a
  per-partition bias AP).

  Phase A: QT (two zero-padded variants, as before, so score matmuls
    contract K=128), KT, and V projected from xT chunks; evacuations
    on DVE write bf16.  V layout per s-block: [V0|1s|V1|V2|1s|V3]
    (384 cols): head h reads a 128-col stationary [V|1] (even h) or
    [1|V] (odd h) with the ones block SHARED between the pair.  The
    M=128 stationary makes the attnV matmul emit 64 IDENTICAL
    softmax-denominator rows on the opposite partition half from the
    64 attn rows - no separate sum-broadcast matmul is needed.
  Phase B per q-chunk of 512: scores transposed s^T[k,q] = K^T Q per
    (head, k-block j); p^T = exp(s^T/8) in ONE ACT per (j, pair)
    reading both heads' PSUM banks ([128,2,W] strided view of a
    2-bank tile); diag blocks get a DVE mask multiply (bf16).
    attn^T[128,q] accumulates matmul(vsb_slice, p^T) over j in PSUM:
    rows 0:64 attn / 64:128 sums for even heads, flipped for odd.
    Normalize: 64-lane reciprocal on the sums half, partition-shift
    DMA (gpsimd queue) to the attn half's partitions, DVE multiply
    writes the bf16 atn tile (pair-packed for phase C).
  Phase C (fused per chunk): out_partial = attn^T.T @ Wo rows with
    K=128 pair contraction; PSUM reuses the psa banks (tag aliasing).
"""

import numpy as np

B, S, D, H, DH = 2, 2048, 1024, 16, 64
HPC = 4                 # heads per core
NCORES = 8
NPAIR = HPC // 2        # head pairs per core (2)
SBLK = S // 128         # 16 s/k blocks
NCH = S // 512          # 4 q-chunks of 512
CHUNKS = D // 128       # 8 contraction chunks of the model dim
EXP_SPLIT = False       # True: one ACT per (j, pair, head) (no 2-bank AP)

_CACHE = {}


def _build_nc(debug=False):
    import concourse.mybir as mybir
    from concourse import bacc, tile

    f32 = mybir.dt.float32
    bf16 = mybir.dt.bfloat16
    AF = mybir.ActivationFunctionType
    OP = mybir.AluOpType

    nc = bacc.Bacc("TRN2", target_bir_lowering=False)

    xT = nc.dram_tensor("xT", [D, S], bf16, kind="ExternalInput")
    wq = nc.dram_tensor("wq", [D, HPC * DH], bf16, kind="ExternalInput")
    wk = nc.dram_tensor("wk", [D, HPC * DH], bf16, kind="ExternalInput")
    wv = nc.dram_tensor("wv", [D, HPC * DH], bf16, kind="ExternalInput")
    wo = nc.dram_tensor("wo", [HPC * DH, D], bf16, kind="ExternalInput")
    # per-pair q/k biases: [128, 4] cols = (q pair0, q pair1, k pair0, k pair1)
    bqk = nc.dram_tensor("bqk", [128, 2 * NPAIR], f32, kind="ExternalInput")
    # bv broadcast to all partitions host-side: [128, 256]
    bvb = nc.dram_tensor("bvb", [128, HPC * DH], f32, kind="ExternalInput")
    # diagonal-block causal masks, duplicated for the two heads of a pair:
    # masks2[k, d, v, f] = (128d + k > f)
    masks2 = nc.dram_tensor("masks2", [128, 4, 2, 512], bf16,
                            kind="ExternalInput")
    out = nc.dram_tensor("out", [S, D], bf16, kind="ExternalOutput")
    if debug:
        qz2_d = nc.dram_tensor("qz2_d", [128, 2, NPAIR, S], bf16,
                               kind="ExternalOutput")
        kt_d = nc.dram_tensor("kt_d", [128, NPAIR, S], bf16,
                              kind="ExternalOutput")
        vsb_d = nc.dram_tensor("vsb_d", [128, SBLK, HPC, 128], bf16,
                               kind="ExternalOutput")
        atn_d = nc.dram_tensor("atn_d", [128, NCH, NPAIR, 512], bf16,
                               kind="ExternalOutput")
        pt_d = nc.dram_tensor("pt_d", [128, NCH, NPAIR, 2, 512], bf16,
                              kind="ExternalOutput")
        psa_d = nc.dram_tensor("psa_d", [128, NCH, HPC, 512], f32,
                               kind="ExternalOutput")

    with tile.TileContext(nc) as tc:
        with (
            tc.tile_pool(name="pers", bufs=1) as pers,
            tc.tile_pool(name="atnp", bufs=2) as atnp,
            tc.tile_pool(name="misc", bufs=1) as misc,
        ):
            # Q^T head pairs, two variants with the other head's rows
            # zeroed so score matmuls can contract K=128.
            qz2 = pers.tile([128, 2, NPAIR, S], bf16)
            kt = pers.tile([128, NPAIR, S], bf16)         # K^T head pairs
            vsb = pers.tile([128, SBLK, HPC, 128], bf16)  # [1s | V] per head
            wo_t = pers.tile([128, NPAIR, D], bf16)       # Wo head pairs
            ones2 = misc.tile([128, 2], bf16)   # [0 | 1] columns
            bias_t = misc.tile([128, 2 * NPAIR], f32)
            bvb_t = misc.tile([128, HPC * DH], f32)
            mask_t = misc.tile([128, 4, 2, 512], bf16)

            nc.gpsimd.memset(ones2[:, 0:1], 0.0)
            nc.gpsimd.memset(ones2[:, 1:2], 1.0)
            # ones block of every [1s | V] stationary: attnV matmuls then
            # emit the softmax denominator on partitions 0:64 of psa
            nc.gpsimd.memset(vsb[:, :, :, 0:DH], 1.0)
            # zero halves of the q variants
            nc.gpsimd.memset(qz2[64:128, 0, :, :], 0.0)
            nc.gpsimd.memset(qz2[0:64, 1, :, :], 0.0)

            # ---------------- Phase A: projections ----------------
            ctxA = nc.named_scope("phaseA"); ctxA.__enter__()
            with (
                tc.tile_pool(name="wts", bufs=1) as wts,
                tc.tile_pool(name="psA", bufs=4, space="PSUM") as psA,
                tc.tile_pool(name="psV", bufs=2, space="PSUM") as psV,
            ):
                xtr = wts.tile([128, CHUNKS, S], bf16)
                wq_t = wts.tile([128, CHUNKS, HPC * DH], bf16, tag="wq")
                wk_t = wts.tile([128, CHUNKS, HPC * DH], bf16, tag="wk")
                wv_t = wts.tile([128, CHUNKS, HPC * DH], bf16, tag="wv")

                # Interleave x and weight chunk loads so chunk c's
                # projections can start as soon as (x_c, w_c) land -
                # queues are FIFO, so emission order is arrival order.
                xT_r = xT.rearrange("(c p) s -> c p s", p=128)
                w_rs = [(w_dram.rearrange("(c p) m -> c p m", p=128), w_t)
                        for w_dram, w_t in
                        ((wq, wq_t), (wk, wk_t), (wv, wv_t))]
                # four parallel DMA queues; x chunk c and its weight
                # chunks land on different queues so they stream in step
                nc.sync.dma_start(bias_t[:], bqk[:])
                nc.scalar.dma_start(bvb_t[:], bvb[:])
                qs = [nc.sync, nc.scalar, nc.gpsimd]
                for c in range(CHUNKS):
                    xe = qs[c % 3]
                    we = qs[(c + 1) % 3]
                    xe.dma_start(xtr[:, c, :], xT_r[c])
                    for w_r, w_tile in w_rs:
                        we.dma_start(w_tile[:, c, :], w_r[c])
                # late-needed loads after the projection inputs
                nc.sync.dma_start(mask_t[:], masks2[:])
                # Wo pairs: rows of pair p = wo[128p : 128p+128]
                wo_r = wo.rearrange("(p r) n -> p r n", r=128)
                for p in range(NPAIR):
                    nc.gpsimd.dma_start(wo_t[:, p, :], wo_r[p])

                # QT / KT: psum[128, 512] accumulated over chunks.
                # Order: KT all, then QT n=0, then V, then QT n>0 -
                # phase B's first q-chunk only needs KT + QT(n=0) + V.
                def proj_group(dsts, p, n):
                    w_tile = wq_t if dsts == "q" else wk_t
                    bcol = (0 if dsts == "q" else NPAIR) + p
                    ps = psA.tile([128, 512], f32, name="ps", tag="ps")
                    for c in range(CHUNKS):
                        nc.tensor.matmul(
                            ps[:],
                            w_tile[:, c, 128 * p:128 * p + 128],
                            xtr[:, c, 512 * n:512 * n + 512],
                            start=(c == 0), stop=(c == CHUNKS - 1))
                    # evacuate on ScalarE (idle during phase A; the DVE is
                    # the busier engine in the steady state)
                    sl = slice(512 * n, 512 * n + 512)
                    bias = bias_t[:, bcol:bcol + 1]
                    if dsts == "k":
                        nc.scalar.activation(kt[:, p, sl], ps[:],
                                             AF.Identity, bias=bias)
                    else:
                        nc.scalar.activation(
                            qz2[0:64, 0, p, sl], ps[0:64, :],
                            AF.Identity, bias=bias[0:64, :])
                        nc.scalar.activation(
                            qz2[64:128, 1, p, sl], ps[64:128, :],
                            AF.Identity, bias=bias[64:128, :])

                for p in range(NPAIR):
                    for n in range(NCH):
                        proj_group("k", p, n)
                for p in range(NPAIR):
                    proj_group("q", p, 0)

                # V: psum[128(s), 256] accumulated over chunks
                bvb_v = bvb_t[:].rearrange("p (h d) -> p h d", h=HPC)
                for sb in range(SBLK):
                    ps = psV.tile([128, HPC * DH], f32)
                    for c in range(CHUNKS):
                        nc.tensor.matmul(
                            ps[:],
                            xtr[:, c, 128 * sb:128 * sb + 128],
                            wv_t[:, c, :],
                            start=(c == 0), stop=(c == CHUNKS - 1))
                    ps_v = ps[:].rearrange("p (h d) -> p h d", h=HPC)
                    nc.vector.tensor_tensor(
                        vsb[:, sb, :, DH:128], ps_v[:], bvb_v[:], op=OP.add)

                for p in range(NPAIR):
                    for n in range(1, NCH):
                        proj_group("q", p, n)
                if debug:
                    nc.sync.dma_start(qz2_d[:], qz2[:])
                    nc.sync.dma_start(kt_d[:], kt[:])
                    nc.sync.dma_start(vsb_d[:], vsb[:])

            ctxA.__exit__(None, None, None)
            # ------------- Phase B + fused C, per q-chunk -------------
            # Software-pipelined: chunk ch's normalize + phase C are
            # emitted INSIDE chunk ch+1's j-loop (after two score/exp
            # iterations, whose attnV matmuls are deferred past the
            # phase-C emissions).  The PE then never head-of-line blocks
            # on the normalize chain, and the ScalarE exp stream restarts
            # immediately at the chunk boundary.
            with (
                tc.tile_pool(name="pt", bufs=6) as ptp,
                tc.tile_pool(name="rcp", bufs=2) as rcpp,
                tc.tile_pool(name="rsh", bufs=2) as rshp,
                tc.tile_pool(name="ob", bufs=4) as obp,
                tc.tile_pool(name="psS", bufs=2, space="PSUM") as psS,
                tc.tile_pool(name="psAt", bufs=1, space="PSUM") as psAt,
            ):
                outq = [nc.sync, nc.gpsimd]

                def emit_scores(ch, j, pair):
                    d = j - 4 * ch
                    W = 128 * (d + 1) if d < 4 else 512
                    pss = psS.tile([128, 1024], f32, tag="pss", name="pss")
                    pt = ptp.tile([128, 2, 512], bf16, name="pt", tag="pt")
                    if d < 2:
                        # both variants in one matmul, stride-256 moving
                        # AP inside bank 0
                        pv = pss[:].rearrange("p (v w) -> p v w", v=4)
                        nc.tensor.matmul(
                            pv[:, 0:2, 0:W],
                            kt[:, pair, 128 * j:128 * j + 128],
                            qz2[:, :, pair, 512 * ch:512 * ch + W],
                            start=True, stop=True)
                        src = pv[:, 0:2, 0:W]
                    else:
                        pv = pss[:].rearrange("p (v w) -> p v w", v=2)
                        for half in range(2):
                            nc.tensor.matmul(
                                pv[:, half, 0:W],
                                kt[:, pair, 128 * j:128 * j + 128],
                                qz2[:, half, pair, 512 * ch:512 * ch + W],
                                start=True, stop=True)
                        src = pv[:, :, 0:W]
                    if EXP_SPLIT and d >= 2:
                        for half in range(2):
                            nc.scalar.activation(pt[:, half, 0:W],
                                                 src[:, half, :],
                                                 AF.Exp, scale=0.125)
                    else:
                        nc.scalar.activation(pt[:, :, 0:W], src,
                                             AF.Exp, scale=0.125)
                    if d < 4:
                        # zero where k <= q
                        nc.vector.tensor_tensor(
                            pt[:, :, 0:W], pt[:, :, 0:W],
                            mask_t[:, d, :, 0:W], op=OP.mult)
                    if debug and j == 4 * ch + 4:
                        nc.sync.dma_start(pt_d[:, ch, pair, :, :], pt[:])
                    return pt

                def make_tail(ch, psa):
                    # normalize: every head has 64 identical sums rows at
                    # partitions 0:64 and attn rows at 64:128.  reciprocal
                    # on 0:64 (the custom DVE op silently no-ops at
                    # base_partition 64), shift up, multiply at 64:128;
                    # even heads (pair position 0:64) take one extra
                    # partition-shift DMA into the atn tile.
                    def tail():
                        atn = atnp.tile([128, NPAIR, 512], bf16,
                                        name="atn")
                        for h in range(HPC):
                            pair, odd = h // 2, h % 2
                            rt = rcpp.tile([128, 512], f32, name="rt")
                            rs = rshp.tile([128, 512], f32, name="rs")
                            nc.vector.reciprocal_approx_fast(
                                rt[0:64, :], psa[h][0:64, :])
                            nc.gpsimd.dma_start(rs[64:128, :], rt[0:64, :])
                            if odd:
                                nc.vector.tensor_tensor(
                                    atn[64:128, pair, :],
                                    psa[h][64:128, :],
                                    rs[64:128, :], op=OP.mult)
                            else:
                                tmp = rshp.tile([128, 512], bf16,
                                                name="tmp", tag="tmp")
                                nc.vector.tensor_tensor(
                                    tmp[64:128, :], psa[h][64:128, :],
                                    rs[64:128, :], op=OP.mult)
                                nc.gpsimd.dma_start(atn[0:64, pair, :],
                                                    tmp[64:128, :])
                            if debug:
                                dps = obp.tile([128, 512], f32,
                                               name="dps", tag="dps")
                                nc.vector.tensor_copy(dps[:], psa[h][:])
                                nc.sync.dma_start(psa_d[:, ch, h, :],
                                                  dps[:])
                        if debug:
                            nc.sync.dma_start(atn_d[:, ch, :, :], atn[:])
                        # fused phase C for this chunk's 4 s-blocks; PSUM
                        # reuses the psa banks via tag aliasing.
                        for k in range(4):
                            sb = 4 * ch + k
                            for n in range(2):
                                ps = psAt.tile([128, 512], f32,
                                               tag=f"psa{k}", name="pso")
                                for p in range(NPAIR):
                                    nc.tensor.matmul(
                                        ps[:],
                                        atn[:, p, 128 * k:128 * k + 128],
                                        wo_t[:, p, 512 * n:512 * n + 512],
                                        start=(p == 0),
                                        stop=(p == NPAIR - 1))
                                ob = obp.tile([128, 512], bf16, name="ob",
                                              tag="ob")
                                nc.vector.tensor_copy(ob[:], ps[:])
                                outq[(2 * k + n) % 2].dma_start(
                                    out[128 * sb:128 * sb + 128,
                                        512 * n:512 * n + 512], ob[:])
                    return tail

                tail_prev = None
                for ch in range(NCH):
                    ctxB = nc.named_scope(f"chunk{ch}"); ctxB.__enter__()
                    js = list(range(4 * ch, SBLK))
                    psa = [None] * HPC
                    deferred = []   # (j, W, pts) awaiting attnV emission

                    def alloc_psa():
                        for h in range(HPC):
                            psa[h] = psAt.tile([128, 512], f32,
                                               tag=f"psa{h}",
                                               name=f"psa{h}")

                    def emit_attnv(j, W, pts, first, last, ch=ch):
                        for h in range(HPC):
                            nc.tensor.matmul(
                                psa[h][:, 0:W],
                                vsb[:, j, h, :],
                                pts[h // 2][:, h % 2, 0:W],
                                start=first, stop=last,
                                skip_group_check=(ch == 3))

                    if tail_prev is None:
                        alloc_psa()
                    for idx, j in enumerate(js):
                        d = j - 4 * ch
                        W = 128 * (d + 1) if d < 4 else 512
                        pts = [emit_scores(ch, j, pair)
                               for pair in range(NPAIR)]
                        last = (idx == len(js) - 1) and ch < 3
                        if tail_prev is not None and idx < 2:
                            deferred.append((j, W, pts))
                            continue
                        if tail_prev is not None:
                            tail_prev(); tail_prev = None
                            alloc_psa()
                            for i, (dj, dW, dpts) in enumerate(deferred):
                                emit_attnv(dj, dW, dpts, i == 0, False)
                            deferred = []
                        emit_attnv(j, W, pts, idx == 0, last)
                    if ch == 3:
                        # last global row q=2047 is fully masked; its exact
                        # value is recomputed on the host.  Keep column
                        # 511's denominator finite (one [0|1]-column
                        # matmul) to avoid Inf/NaN noise.
                        for h in range(HPC):
                            nc.tensor.matmul(
                                psa[h][:, 510:512],
                                vsb[:, 0, h, :], ones2[:],
                                start=False, stop=True)
                    tail_prev = make_tail(ch, psa)
                    ctxB.__exit__(None, None, None)
                tail_prev()

    nc.finalize()
    return nc


def _prep_in_maps(inputs, Wq, bq, Wk, bk, Wv, bv, Wo, bo):
    import ml_dtypes
    bfdt = ml_dtypes.bfloat16
    in_maps = []
    xTs = [np.ascontiguousarray(inputs[b].T.astype(bfdt)) for b in range(B)]
    kk = np.arange(128)[:, None, None, None]
    dd = np.arange(4)[None, :, None, None]
    ff = np.arange(512)[None, None, None, :]
    masks2 = ((128 * dd + kk) > ff).astype(bfdt)
    masks2 = np.ascontiguousarray(np.broadcast_to(masks2, (128, 4, 2, 512)))
    for core in range(NCORES):
        b = core // (NCORES // B)
        g = core % (NCORES // B)
        cols = slice(g * HPC * DH, (g + 1) * HPC * DH)
        bq_c = bq[cols].reshape(NPAIR, 128).T          # [128, 2]
        bk_c = bk[cols].reshape(NPAIR, 128).T
        bqk_c = np.ascontiguousarray(
            np.concatenate([bq_c, bk_c], axis=1), dtype=np.float32)
        bvb_c = np.ascontiguousarray(
            np.broadcast_to(bv[cols][None, :], (128, HPC * DH)),
            dtype=np.float32)
        in_maps.append({
            "xT": xTs[b],
            "wq": np.ascontiguousarray(Wq[:, cols].astype(bfdt)),
            "wk": np.ascontiguousarray(Wk[:, cols].astype(bfdt)),
            "wv": np.ascontiguousarray(Wv[:, cols].astype(bfdt)),
            "wo": np.ascontiguousarray(Wo[cols, :].astype(bfdt)),
            "bqk": bqk_c,
            "bvb": bvb_c,
            "masks2": masks2,
        })
    return in_maps


def kernel(inputs, Wq, bq, Wk, bk, Wv, bv, Wo, bo, _want_results=False,
           _debug=False, **_run_kwargs):
    from concourse.bass_utils import run_bass_kernel_spmd

    inputs = np.asarray(inputs, dtype=np.float32)
    Wq, bq = np.asarray(Wq, np.float32), np.asarray(bq, np.float32)
    Wk, bk = np.asarray(Wk, np.float32), np.asarray(bk, np.float32)
    Wv, bv = np.asarray(Wv, np.float32), np.asarray(bv, np.float32)
    Wo, bo = np.asarray(Wo, np.float32), np.asarray(bo, np.float32)

    key = "nc_dbg" if _debug else "nc"
    if key not in _CACHE:
        _CACHE[key] = _build_nc(debug=_debug)
    nc = _CACHE[key]

    in_maps = _prep_in_maps(inputs, Wq, bq, Wk, bk, Wv, bv, Wo, bo)
    res = run_bass_kernel_spmd(nc, in_maps, core_ids=list(range(NCORES)),
                               **_run_kwargs)

    out = np.zeros((B, S, D), dtype=np.float32)
    for core in range(NCORES):
        b = core // (NCORES // B)
        out[b] += res.results[core]["out"].astype(np.float32)
    out += bo[None, None, :]
    # exact last row (fully masked -> uniform attention = mean(V) @ Wo)
    for b in range(B):
        v_mean = inputs[b].mean(axis=0) @ Wv + bv
        out[b, S - 1, :] = v_mean @ Wo + bo
    if _want_results:
        return out, res
    return out
